# revision 2
# baseline (speedup 1.0000x reference)
"""Multi-head attention (B=4, S=2048, D=1280, H=10, hd=128) on 8 TRN2 NeuronCores.

Sharding: core c handles batch b = c//2 and heads h0 = 5*(c%2) .. h0+5.
Per core:
  qkvT = wqkv_c^T @ x_b^T          (Q^T, K^T in SBUF, V transposed to token-major)
  per head: S^T = K_h Q_h^T * scale (via PE), P = exp(S^T) (ACT, no max-sub:
            scores are tiny because scale = D**-0.5), colsums via ones-matmul,
            O'^T = V_h^T P^T (PE accum), normalize by 1/colsum (broadcast matmul)
  outT_partial = wout_c^T @ O^T    (row-sharded out projection -> partial sums)
Host: out[b] = outT_{2b}.T + outT_{2b+1}.T + b_out  (all-reduce done on host).

All matmuls run in float32r (TF32-like fast mode, 1 cycle/row at N>=256).
"""

import numpy as np

B, S, D = 4, 2048, 1280
HEADS = 10
HD = 128
NH = 5              # heads per core
P = 128
SCALE = float(D) ** -0.5
KT_D = D // P       # 10 k-tiles over D
MT = 3 * NH         # 15 m-tiles over local qkv dim (Q 0-4, K 5-9, V 10-14)
NJT = S // P        # 16 j tiles
NIC = S // 512      # 4 i-chunks of 512
QKV_NIC = S // 256  # 8 sub-chunks of 256 for the qkv projection (SBUF economy)

_PROGRAM_CACHE = {}


def _build_program(repeat=1):
    if repeat in _PROGRAM_CACHE:
        return _PROGRAM_CACHE[repeat]

    import concourse.bass as bass  # noqa: F401
    import concourse.mybir as mybir
    from concourse import bacc
    import concourse.tile as tile
    from concourse.masks import make_identity

    F32 = mybir.dt.float32
    F32R = mybir.dt.float32r
    EXP = mybir.ActivationFunctionType.Exp

    nc = bacc.Bacc()
    xT_d = nc.declare_dram_parameter("xT", [D, S], F32, isOutput=False)
    wqkv_d = nc.declare_dram_parameter("wqkv", [D, 3 * NH * HD], F32, isOutput=False)
    wout_d = nc.declare_dram_parameter("wout", [NH * HD, D], F32, isOutput=False)
    ones_d = nc.declare_dram_parameter("ones_in", [P, 1], F32, isOutput=False)
    onesr_d = nc.declare_dram_parameter("onesr_in", [1, P], F32, isOutput=False)
    out_d = nc.declare_dram_parameter("outT", [D, S], F32, isOutput=True)

    xT_t = xT_d[:].rearrange("(kt p) s -> p kt s", p=P)          # [128, 10, 2048]
    wqkv_t = wqkv_d[:].rearrange("(kt p) m -> p kt m", p=P)      # [128, 10, 1920]
    wout_t = wout_d[:].rearrange("(kt p) m -> p kt m", p=P)      # [128, 5, 1280]

    with tile.TileContext(nc) as tc:
        with (
            tc.tile_pool(name="persist", bufs=1) as persist,
            tc.tile_pool(name="io", bufs=2) as io,
            tc.tile_pool(name="oio", bufs=1) as oio,
            tc.tile_pool(name="wpool", bufs=2) as wpool,
            tc.tile_pool(name="work", bufs=3) as work,
            tc.tile_pool(name="work2", bufs=2) as work2,
            tc.tile_pool(name="ps_mm", bufs=3, space="PSUM") as ps_mm,
            tc.tile_pool(name="ps_acc", bufs=2, space="PSUM") as ps_acc,
            tc.tile_pool(name="ps_one", bufs=1, space="PSUM") as ps_one,
            tc.tile_pool(name="ps_bc", bufs=1, space="PSUM") as ps_bc,
            tc.tile_pool(name="ps_tp", bufs=1, space="PSUM") as ps_tp,
        ):
            QT = persist.tile([P, NH, S], F32R, name="QT")
            KT = persist.tile([P, NH, S], F32R, name="KT")
            V = persist.tile([P, NJT, NH, HD], F32R, name="V")
            ones = persist.tile([P, 1], F32R, name="ones")
            onesr = persist.tile([1, P], F32R, name="onesr")
            ident = persist.tile([P, P], F32, name="ident")

            nc.sync.dma_start(ones[:], ones_d[:].bitcast(F32R))
            nc.sync.dma_start(onesr[:], onesr_d[:].bitcast(F32R))
            make_identity(nc, ident[:])

            for rep in range(repeat):
                # ---------------- Phase 1: QKV projection ----------------
                for ic in range(QKV_NIC):
                    isl = slice(ic * 256, (ic + 1) * 256)
                    xt = io.tile([P, KT_D, 256], F32R, name="xt")
                    nc.sync.dma_start(xt[:], xT_t[:, :, isl].bitcast(F32R))
                    for m in range(MT):
                        wt = wpool.tile([P, KT_D, P], F32R, name="wt")
                        nc.sync.dma_start(
                            wt[:], wqkv_t[:, :, m * P:(m + 1) * P].bitcast(F32R)
                        )
                        q_ps = ps_mm.tile([P, 512], F32, name="mm")[:, :256]
                        for kt in range(KT_D):
                            nc.tensor.matmul(
                                q_ps, wt[:, kt, :], xt[:, kt, :],
                                start=(kt == 0), stop=(kt == KT_D - 1),
                            )
                        if m < NH:  # Q
                            nc.scalar.copy(QT[:, m, isl], q_ps)
                        elif m < 2 * NH:  # K
                            nc.scalar.copy(KT[:, m - NH, isl], q_ps)
                        else:  # V: psum holds V^T slice [hd, 256 tokens]
                            h = m - 2 * NH
                            vt = work.tile([P, 256], F32, name="vt")
                            nc.scalar.copy(vt[:], q_ps)
                            for tt in range(2):
                                jt = ic * 2 + tt
                                t_ps = ps_tp.tile([P, P], F32, name="tp")
                                nc.tensor.transpose(
                                    t_ps[:], vt[:, tt * P:(tt + 1) * P], ident[:]
                                )
                                nc.scalar.copy(V[:, jt, h, :], t_ps[:])

                # ------------- Phase 2: attention + out projection -------------
                for ic in range(NIC):
                    isl = slice(ic * 512, (ic + 1) * 512)
                    OT = oio.tile([P, NH, 512], F32R, name="OT")
                    for h in range(NH):
                        fold = work2.tile([P, 512], F32R, name="fold")
                        o_ps = ps_acc.tile([P, 512], F32, name="acc")
                        for jt in range(NJT):
                            s_ps = ps_mm.tile([P, 512], F32, name="mm")
                            nc.tensor.matmul(
                                s_ps[:], KT[:, h, jt * P:(jt + 1) * P],
                                QT[:, h, isl], start=True, stop=True,
                            )
                            pt = work.tile([P, 512], F32R, name="pt")
                            nc.scalar.activation(pt[:], s_ps[:], EXP, scale=SCALE)
                            if jt == 0:
                                nc.vector.tensor_copy(fold[:], pt[:])
                            else:
                                nc.vector.tensor_add(fold[:], fold[:], pt[:])
                            nc.tensor.matmul(
                                o_ps[:], V[:, jt, h, :], pt[:],
                                start=(jt == 0), stop=(jt == NJT - 1),
                            )
                        # column sums -> broadcast reciprocal -> normalize
                        sum_ps = ps_one.tile([1, 512], F32, name="one")
                        nc.tensor.matmul(sum_ps[:], ones[:], fold[:],
                                         start=True, stop=True)
                        s_row = work2.tile([1, 512], F32R, name="s_row")
                        nc.scalar.copy(s_row[:], sum_ps[:])
                        bc_ps = ps_bc.tile([P, 512], F32, name="bc")
                        nc.tensor.matmul(bc_ps[:], onesr[:], s_row[:],
                                         start=True, stop=True)
                        rec = work2.tile([P, 512], F32, name="rec")
                        nc.vector.reciprocal(rec[:], bc_ps[:])
                        nc.vector.tensor_mul(OT[:, h, :], o_ps[:], rec[:])

                    for m in range(D // P):
                        wo = wpool.tile([P, NH, P], F32R, name="wo")
                        nc.sync.dma_start(
                            wo[:], wout_t[:, :, m * P:(m + 1) * P].bitcast(F32R)
                        )
                        p_ps = ps_mm.tile([P, 512], F32, name="mm")
                        for kt in range(NH):
                            nc.tensor.matmul(
                                p_ps[:], wo[:, kt, :], OT[:, kt, :],
                                start=(kt == 0), stop=(kt == NH - 1),
                            )
                        outc = work.tile([P, 512], F32, name="outc")
                        nc.scalar.copy(outc[:], p_ps[:])
                        nc.sync.dma_start(out_d[m * P:(m + 1) * P, isl], outc[:])

    nc.finalize()
    _PROGRAM_CACHE[repeat] = nc
    return nc


def _shard_inputs(x, w_qkv, w_out):
    """Build the 8 per-core input maps."""
    ones = np.ones((P, 1), np.float32)
    onesr = np.ones((1, P), np.float32)
    in_maps = []
    for c in range(8):
        b = c // 2
        h0 = NH * (c % 2)
        cols = np.concatenate([
            w_qkv[:, qi * D + h0 * HD: qi * D + (h0 + NH) * HD] for qi in range(3)
        ], axis=1)                                   # [D, 1920]
        in_maps.append(dict(
            xT=np.ascontiguousarray(x[b].T),          # [D, S]
            wqkv=np.ascontiguousarray(cols),          # [D, 1920]
            wout=np.ascontiguousarray(w_out[h0 * HD:(h0 + NH) * HD, :]),  # [640, D]
            ones_in=ones,
            onesr_in=onesr,
        ))
    return in_maps


def run_sharded(x, w_qkv, w_out, b_out, repeat=1, trace=False):
    """Run the SPMD program; returns (out [B,S,D], BassKernelResults)."""
    from concourse.bass_utils import run_bass_kernel_spmd

    nc = _build_program(repeat)
    in_maps = _shard_inputs(x, w_qkv, w_out)
    res = run_bass_kernel_spmd(nc, in_maps, list(range(8)), trace=trace)
    out = np.empty((B, S, D), np.float32)
    for b in range(B):
        out[b] = (res.results[2 * b]["outT"].T
                  + res.results[2 * b + 1]["outT"].T
                  + b_out[None, :])
    return out, res


def kernel(x, w_qkv, w_out, b_out):
    x = np.asarray(x, np.float32)
    w_qkv = np.asarray(w_qkv, np.float32)
    w_out = np.asarray(w_out, np.float32)
    b_out = np.asarray(b_out, np.float32)
    out, _ = run_sharded(x, w_qkv, w_out, b_out)
    return out
